# revision 20
# baseline (speedup 1.0000x reference)
"""Multi-head attention (B=4, S=2048, D=1024, H=16, dk=dv=64) on 8 Trainium2
NeuronCores.

Sharding: core c handles batch b = c//2 and heads (c%2)*8 .. (c%2)*8+8
(data parallel on batch x tensor parallel on heads). Attention is head-local;
the output-projection partial sums of the two cores sharing a batch are
reduced on the host.

Device-side layout trick: scores are computed TRANSPOSED (S^T[tk, tq]) so the
same tensor feeds both the softmax normalization (column sums obtained for
free through a ones-column appended to the V projection) and the
attn @ V matmul (which contracts over tk = partitions). The attention
probabilities are therefore written to DRAM transposed per head and
transposed back on the host during unsharding.

Bias handling (exact, not approximate):
  - bk shifts every score of a query row by a constant -> softmax invariant;
    dropped.
  - bq adds bq.kh[tk] to scores: folded into the Exp bias (per-partition) via
    an extra tiny projection sadj = kT @ (Wk_h @ bq_h) computed on device
    (only emitted when bq != 0).
  - bv, bo: attn rows sum to 1, so their effect is the constant vector
    bv @ Wo + bo added on the host.
"""

import sys

import numpy as np

if "/opt/trn_rl_repo" not in sys.path:
    sys.path.insert(0, "/opt/trn_rl_repo")

B, S, D = 4, 2048, 1024
H, DK, DV = 16, 64, 64
NCORES = 8
HPC = H // 2  # heads per core (8)


def _emit(nc, tc, io, cfg):
    """Emit the Tile program for one core (SPMD: all cores run this)."""
    import concourse.mybir as mybir

    f32 = mybir.dt.float32
    f32r = mybir.dt.float32r
    Exp = mybir.ActivationFunctionType.Exp

    S_, D_, HPC_ = cfg["S"], cfg["D"], cfg["HPC"]
    use_sadj = cfg["use_sadj"]
    KC = D_ // 128          # contraction chunks for projections
    NTQ = S_ // 512         # tq chunks
    NTK = S_ // 128         # tk (= token) tiles
    MT = HPC_ * DK // 128   # head-dim tiles (2 heads each)
    GROUP = 1 if use_sadj else 2  # tk-tiles per Exp instruction
    NG = NTK // GROUP
    HD = HPC_ * DK          # per-core projected width
    DCH = max(1, D_ // 512)  # output-projection d chunks
    DSZ = D_ // DCH

    pers_cm = tc.tile_pool(name="pers", bufs=1)
    pers = pers_cm.__enter__()
    qhT = pers.tile([128, MT, S_], f32r)            # [hd%128, hd-tile, tok]
    khT = pers.tile([128, MT, S_], f32r)
    vh = pers.tile([128, NTK, HPC_, DV + 1], f32r)  # [tok%128, tok-tile, h, dv|1]
    sadjT = pers.tile([128, NTK, HPC_], f32, name="sadjT") if use_sadj else None

    # ---------------- Phase A: projections ----------------
    with (
        tc.tile_pool(name="wsb", bufs=1) as wsb,
        tc.tile_pool(name="astream", bufs=3) as astream,
        tc.tile_pool(name="prps", bufs=4, space="PSUM") as prps,
        tc.tile_pool(name="sadjps", bufs=4, space="PSUM") as sadjps,
    ):
        wq = wsb.tile([128, KC, HPC_ * DK], f32r)
        wk = wsb.tile([128, KC, HPC_ * DK], f32r)
        wv = wsb.tile([128, KC, HPC_ * DK], f32r)
        nc.sync.dma_start(wq[:], io["wq"].rearrange("(c p) n -> p c n", p=128))
        nc.sync.dma_start(wk[:], io["wk"].rearrange("(c p) n -> p c n", p=128))
        nc.sync.dma_start(wv[:], io["wv"].rearrange("(c p) n -> p c n", p=128))
        if use_sadj:
            wqt = wsb.tile([128, KC, HPC_], f32r)
            nc.sync.dma_start(wqt[:], io["wqt"].rearrange("(c p) n -> p c n", p=128))
        nc.vector.memset(vh[:].bitcast(f32), 1.0)  # pre-sets the ones column (rest overwritten)

        for t in range(NTQ):
            tok = slice(t * 512, (t + 1) * 512)
            # --- q -> qhT
            psq = [prps.tile([128, 512], f32, tag="prj", name="psq") for _ in range(MT)]
            for c in range(KC):
                qc = astream.tile([128, 512], f32r, tag="qc")
                nc.sync.dma_start(qc[:], io["qT"][c * 128:(c + 1) * 128, tok])
                for m in range(MT):
                    nc.tensor.matmul(
                        psq[m][:],
                        wq[:, c, m * 128:(m + 1) * 128],
                        qc[:],
                        start=(c == 0), stop=(c == KC - 1),
                    )
            for m in range(MT):
                nc.scalar.copy(qhT[:, m, tok], psq[m][:])
            # --- k -> khT (+ sadj)
            psk = [prps.tile([128, 512], f32, tag="prj", name="psq") for _ in range(MT)]
            if use_sadj:
                pss = [sadjps.tile([128, HPC_], f32, tag="sadj", name="pss") for _ in range(4)]
            for c in range(KC):
                kc = astream.tile([128, 512], f32r, tag="kc")
                nc.sync.dma_start(kc[:], io["kT"][c * 128:(c + 1) * 128, tok])
                for m in range(MT):
                    nc.tensor.matmul(
                        psk[m][:],
                        wk[:, c, m * 128:(m + 1) * 128],
                        kc[:],
                        start=(c == 0), stop=(c == KC - 1),
                    )
                if use_sadj:
                    for j in range(4):
                        nc.tensor.matmul(
                            pss[j][:],
                            kc[:, j * 128:(j + 1) * 128],
                            wqt[:, c, :],
                            start=(c == 0), stop=(c == KC - 1),
                        )
            for m in range(MT):
                nc.scalar.copy(khT[:, m, tok], psk[m][:])
            if use_sadj:
                for j in range(4):
                    nc.scalar.copy(sadjT[:, t * 4 + j, :], pss[j][:])
            # --- v -> vh
            psv = [prps.tile([128, HD], f32, tag="prj", name="psv") for _ in range(4)]
            for c in range(KC):
                vc = astream.tile([128, 512], f32r, tag="vc")
                nc.sync.dma_start(vc[:], io["vT"][c * 128:(c + 1) * 128, tok])
                for j in range(4):
                    nc.tensor.matmul(
                        psv[j][:],
                        vc[:, j * 128:(j + 1) * 128],
                        wv[:, c, :],
                        start=(c == 0), stop=(c == KC - 1),
                    )
            for j in range(4):
                nc.scalar.copy(
                    vh[:, t * 4 + j, :, 0:DV],
                    psv[j][:].rearrange("p (h d) -> p h d", h=HPC_),
                )

    # ---------------- Phase B: attention + output projection ----------------
    with (
        tc.tile_pool(name="wo_sb", bufs=1) as wop,
        tc.tile_pool(name="upool", bufs=2) as upool,
        tc.tile_pool(name="ostp", bufs=1) as ostp,
        tc.tile_pool(name="rows", bufs=2) as rows,
        tc.tile_pool(name="postage", bufs=2) as postage,
        tc.tile_pool(name="tpsum", bufs=2, space="PSUM") as tpsum,
        tc.tile_pool(name="pvpo", bufs=3, space="PSUM") as pvpo,
        tc.tile_pool(name="bps", bufs=1, space="PSUM") as bps,
    ):
        wo = wop.tile([128, MT, D_], f32r)
        nc.sync.dma_start(wo[:], io["wo"].rearrange("(m p) d -> p m d", p=128))
        ones = wop.tile([1, 128], f32)
        nc.vector.memset(ones[:], 1.0)

        for t in range(NTQ):
            tq = slice(t * 512, (t + 1) * 512)
            ost = ostp.tile([128, MT, 512], f32r)
            for h in range(HPC_):
                m, hp = h // 2, (h % 2) * 64
                qsl = qhT[hp:hp + 64, m, tq]
                pv = pvpo.tile([DV + 1, 512], f32, tag="pvpo")
                ut = upool.tile([128, NTK, 512], f32r)
                for g in range(NG):
                    tp = tpsum.tile([128, GROUP, 512], f32, tag="tp")
                    for s in range(GROUP):
                        tk = g * GROUP + s
                        nc.tensor.matmul(
                            tp[:, s, :],
                            khT[hp:hp + 64, m, tk * 128:(tk + 1) * 128],
                            qsl,
                            start=True, stop=True,
                        )
                    if use_sadj:
                        nc.scalar.activation(
                            ut[:, g * GROUP:(g + 1) * GROUP, :], tp[:],
                            Exp, bias=sadjT[:, g, h:h + 1], scale=0.125,
                        )
                    else:
                        nc.scalar.activation(
                            ut[:, g * GROUP:(g + 1) * GROUP, :], tp[:],
                            Exp, scale=0.125,
                        )
                    for s in range(GROUP):
                        tk = g * GROUP + s
                        nc.tensor.matmul(
                            pv[:],
                            vh[:, tk, h, :],
                            ut[:, tk, :],
                            start=(tk == 0), stop=(tk == NTK - 1),
                        )
                # softmax denominator: row DV of pv is sum_tk exp(scores)
                cs = rows.tile([1, 512], f32, tag="cs")
                nc.scalar.copy(cs[:], pv[DV:DV + 1, :])
                rr = rows.tile([1, 512], f32, tag="rr")
                nc.vector.reciprocal_approx_fast(rr[:], cs[:])
                bb = bps.tile([128, 512], f32)
                nc.tensor.matmul(bb[:], ones[:], rr[:], start=True, stop=True)
                bbs = rows.tile([128, 512], f32, tag="bbs")
                nc.scalar.copy(bbs[:], bb[:])
                for a in range(NTK):
                    nc.vector.tensor_mul(ut[:, a, :], ut[:, a, :], bbs[:])
                nc.sync.dma_start(
                    io["attn_t"][h].rearrange("(a p) q -> p a q", p=128)[:, :, tq],
                    ut[:],
                )
                nc.vector.tensor_mul(ost[hp:hp + 64, m, :], pv[0:DV, :], bbs[0:DV, :])
            # output projection for this tq chunk
            for j in range(4):
                for dc in range(DCH):
                    pp = pvpo.tile([128, DSZ], f32, tag="pvpo")
                    for mm in range(MT):
                        nc.tensor.matmul(
                            pp[:],
                            ost[:, mm, j * 128:(j + 1) * 128],
                            wo[:, mm, dc * DSZ:(dc + 1) * DSZ],
                            start=(mm == 0), stop=(mm == MT - 1),
                        )
                    pc = postage.tile([128, DSZ], f32, tag="pc")
                    nc.scalar.copy(pc[:], pp[:])
                    r0 = (t * 4 + j) * 128
                    nc.sync.dma_start(
                        io["po"][r0:r0 + 128, dc * DSZ:(dc + 1) * DSZ], pc[:]
                    )

    pers_cm.__exit__(None, None, None)


def _build(cfg):
    import concourse.bacc as bacc
    import concourse.mybir as mybir
    import concourse.tile as tile

    f32 = mybir.dt.float32
    f32r = mybir.dt.float32r
    S_, D_, HPC_ = cfg["S"], cfg["D"], cfg["HPC"]
    nc = bacc.Bacc("TRN2", target_bir_lowering=False, debug=False)
    io = {
        "qT": nc.dram_tensor("qT", [D_, S_], f32r, kind="ExternalInput").ap(),
        "kT": nc.dram_tensor("kT", [D_, S_], f32r, kind="ExternalInput").ap(),
        "vT": nc.dram_tensor("vT", [D_, S_], f32r, kind="ExternalInput").ap(),
        "wq": nc.dram_tensor("wq", [D_, HPC_ * DK], f32r, kind="ExternalInput").ap(),
        "wk": nc.dram_tensor("wk", [D_, HPC_ * DK], f32r, kind="ExternalInput").ap(),
        "wv": nc.dram_tensor("wv", [D_, HPC_ * DV], f32r, kind="ExternalInput").ap(),
        "wo": nc.dram_tensor("wo", [HPC_ * DV, D_], f32r, kind="ExternalInput").ap(),
        "attn_t": nc.dram_tensor("attn_t", [HPC_, S_, S_], f32r, kind="ExternalOutput").ap(),
        "po": nc.dram_tensor("po", [S_, D_], f32, kind="ExternalOutput").ap(),
    }
    if cfg["use_sadj"]:
        io["wqt"] = nc.dram_tensor("wqt", [D_, HPC_], f32r, kind="ExternalInput").ap()
    with tile.TileContext(nc) as tc:
        _emit(nc, tc, io, cfg)
    nc.compile()
    return nc


_cache = {}
_TRACE = False
_last_exec_ns = None


def _get_nc(use_sadj):
    key = bool(use_sadj)
    if key not in _cache:
        _cache[key] = _build(
            {"S": S, "D": D, "HPC": HPC, "use_sadj": key}
        )
    return _cache[key]


def kernel(q, k, v, Wq, bq, Wk, bk, Wv, bv, Wo, bo):
    from concourse.bass_utils import run_bass_kernel_spmd

    q = np.asarray(q, np.float32)
    k = np.asarray(k, np.float32)
    v = np.asarray(v, np.float32)
    Wq = np.asarray(Wq, np.float32)
    Wk = np.asarray(Wk, np.float32)
    Wv = np.asarray(Wv, np.float32)
    Wo = np.asarray(Wo, np.float32)
    bq = np.asarray(bq, np.float32)
    bk = np.asarray(bk, np.float32)
    bv = np.asarray(bv, np.float32)
    bo = np.asarray(bo, np.float32)

    use_sadj = bool(np.any(bq != 0.0))
    nc = _get_nc(use_sadj)

    in_maps = []
    for c in range(NCORES):
        b, hh = c // 2, (c % 2) * HPC
        hs = slice(hh * DK, (hh + HPC) * DK)
        m = {
            "qT": np.ascontiguousarray(q[b].T),
            "kT": np.ascontiguousarray(k[b].T),
            "vT": np.ascontiguousarray(v[b].T),
            "wq": np.ascontiguousarray(Wq[:, hs]),
            "wk": np.ascontiguousarray(Wk[:, hs]),
            "wv": np.ascontiguousarray(Wv[:, hs]),
            "wo": np.ascontiguousarray(Wo[hs, :]),
        }
        if use_sadj:
            wqt = np.empty((D, HPC), np.float32)
            for h in range(HPC):
                g = hh + h
                wqt[:, h] = Wk[:, g * DK:(g + 1) * DK] @ bq[g * DK:(g + 1) * DK]
            m["wqt"] = 0.125 * wqt
        in_maps.append(m)

    global _last_exec_ns
    res = run_bass_kernel_spmd(
        nc, in_maps, core_ids=list(range(NCORES)), trace=_TRACE
    )
    _last_exec_ns = res.exec_time_ns
    results = res.results

    const = (bv @ Wo + bo).astype(np.float32)
    out = np.empty((B, S, D), np.float32)
    attn = np.empty((B, H, S, S), np.float32)
    for b in range(B):
        out[b] = results[2 * b]["po"] + results[2 * b + 1]["po"] + const
        attn[b, 0:HPC] = results[2 * b]["attn_t"].transpose(0, 2, 1)
        attn[b, HPC:H] = results[2 * b + 1]["attn_t"].transpose(0, 2, 1)
    return out, attn
